# revision 3
# baseline (speedup 1.0000x reference)
"""Trainium2 Bass kernel for nn_LIFcomplexLayer (v2).

Sharding: (B x H) = 2 x 4 across 8 cores: core c handles batches
[(c%2)*16, +16) and neurons [(c//2)*128, +128). With only 128 neurons per
core, every per-neuron constant is a per-partition [P,1] scalar, so the
serial recurrence runs as 4 fused scalar_tensor_tensor ops per step
(2 on Pool, 2 on DVE) over flat [128,16] tiles.

Recurrence algebra (alpha = a_r + i*a_i, state w = -a_i*ui, v = w + d_next):
  negm_t = (r_{t-1} > 0.5) - r_{t-1}          (= s_{t-1} - r_{t-1})
  r_t    = -a_r * negm_t + v_{t-1}
  v_t    = a_i^2 * negm_t + a_r * v_{t-1} + e_t
  e_t    = d_{t+1} - a_r * d_t               (precomputed in bulk)
where d = BN(Wx)*gamma*b + beta*b is folded as
  e_t = gsc*(Wx_{t+1} - a_r*Wx_t) + hof*(1 - a_r),
  gsc = b*gamma*invstd, hof = b*beta - mean*gsc.

Phase A: host pre-transposes x to [b, i, t]; device streams it and runs
fp32r matmuls (1 cyc/row) against W^T, accumulating raw Wx in a resident
SBUF buffer [128h, 16b, 2048t] plus per-neuron sum/sumsq for BN.
Phase B: stats AllReduce across the 2 cores sharing each h-slice, BN
finalize, chunked in-place e-transform, v_{-1} init.
Phase C: 2048-step loop; r_t is written into wxbuf column (t-1) mod 2048
(the e column consumed one step earlier), raw membrane values are DMA'd
out in 256-col chunks overlapping the loop; host applies the spike
threshold (bit-identical in f32) and un-rotates the column mapping.
"""

import sys

if "/opt/trn_rl_repo" not in sys.path:
    sys.path.insert(0, "/opt/trn_rl_repo")

import os
import numpy as np

B, T, I, H = 32, 2048, 512, 512
NCORES = 8
P = 128
BLOC = 16                   # batches per core
HLOC = 128                  # neurons per core
IC = I // P                 # 4 i-chunks
NTOT = float(B * T)
ECH = 32                    # e-transform chunk width (16b x 32 = 512 PSUM f32)
OCH = 256                   # output DMA chunk width

TSTEPS = int(os.environ.get("LIF_TSTEPS", str(T)))

_CACHE = {}


def _register_lif_ops():
    import numpy as np
    from concourse.dve_spec import Spec, Src0, Src1, C0, C1, lower, _has_src1
    from concourse.dve_uop import DveOpSpec
    from concourse.dve_ops import (
        DveOp, OPS, CUSTOM_DVE_SPECS, _SUB_OPCODE_FOR_NAME,
        _CUSTOM_DVE_ROW_BASE,
    )

    for op in OPS:
        if op.name == "LIFG_ANT":
            return op

    def mk(name, body, ref):
        spec = Spec(body=body, reference=ref)
        row = _CUSTOM_DVE_ROW_BASE + len(OPS)
        uops = lower(spec, ver="v3")
        tmp = DveOpSpec(name=name, opcode=row, uops=uops, rd1_en=_has_src1(spec))
        sha = tmp.sha("v3")
        op = DveOp(name, spec, subdim=False, uops_sha={"v3": sha})
        OPS.append(op)
        CUSTOM_DVE_SPECS[name] = spec
        _SUB_OPCODE_FOR_NAME[name] = row
        return op

    from concourse.dve_spec import C2
    s0gt = Src0 > C2
    lifg = mk(
        "LIFG_ANT", (s0gt - Src0) * C0 + Src1 * C1,
        lambda in0, in1, s0, s1, imm2:
            (((in0 > imm2).astype(np.float32)) - in0) * s0 + in1 * s1,
    )
    return lifg


def _build():
    import concourse.bass as bass
    import concourse.bacc as bacc
    import concourse.tile as tile
    from concourse import mybir
    from contextlib import ExitStack

    dt = mybir.dt
    f32 = dt.float32
    Alu = mybir.AluOpType
    Act = mybir.ActivationFunctionType

    LIFG = _register_lif_ops()

    nc = bacc.Bacc(
        "TRN2", target_bir_lowering=False, debug=False, num_devices=NCORES
    )

    # x pre-transposed on host: [b, i, t]
    x_d = nc.dram_tensor("x", [BLOC, I, T], f32, kind="ExternalInput").ap()
    wt_d = nc.dram_tensor("wt", [P, IC, HLOC], f32, kind="ExternalInput").ap()
    cons_d = nc.dram_tensor("cons", [P, 6], f32, kind="ExternalInput").ap()
    st0_d = nc.dram_tensor("st0", [P, 2, BLOC], f32, kind="ExternalInput").ap()
    out_d = nc.dram_tensor("out", [P, BLOC, T], f32, kind="ExternalOutput").ap()
    out0_d = nc.dram_tensor("out0", [P, BLOC], f32, kind="ExternalOutput").ap()

    with tile.TileContext(nc) as tc, ExitStack() as ctx:
        consts = ctx.enter_context(tc.tile_pool(name="consts", bufs=1))
        big = ctx.enter_context(tc.tile_pool(name="big", bufs=1))
        xin = ctx.enter_context(tc.tile_pool(name="xin", bufs=2))
        mpool = ctx.enter_context(tc.tile_pool(name="psumM", bufs=4, space="PSUM"))
        small = ctx.enter_context(tc.tile_pool(name="small", bufs=1))
        scr = ctx.enter_context(tc.tile_pool(name="scr", bufs=2))
        state = ctx.enter_context(tc.tile_pool(name="state", bufs=1))
        dram = ctx.enter_context(tc.tile_pool(name="dram", bufs=1, space="DRAM"))

        wt_sb = consts.tile([P, IC, HLOC], f32)
        nc.sync.dma_start(wt_sb[:], wt_d[:])
        cons_sb = consts.tile([P, 6], f32)
        nc.sync.dma_start(cons_sb[:], cons_d[:])
        st0_sb = consts.tile([P, 2, BLOC], f32)
        nc.sync.dma_start(st0_sb[:], st0_d[:])

        nar = cons_sb[:, 0:1]
        arr = cons_sb[:, 1:2]
        aisq = cons_sb[:, 2:3]
        bg = cons_sb[:, 3:4]
        bb = cons_sb[:, 4:5]
        aa2 = cons_sb[:, 5:6]

        # Raw Wx, then e, then membrane values r (shifted one column).
        wxbuf = big.tile([P, BLOC, T], f32)
        NST = BLOC * (T // 512)  # stat columns: one per (b, 512-t chunk)
        sumS = small.tile([P, NST], f32)
        sumQ = small.tile([P, NST], f32)

        # ---- phase A: stream xT, fp32r matmuls, raw Wx + stats ----
        for b in range(BLOC):
            for half in range(2):
                xt = xin.tile([P, IC, T // 2], f32)
                nc.sync.dma_start(
                    xt[:],
                    x_d[b, :, half * (T // 2) : (half + 1) * (T // 2)].rearrange(
                        "(ic p) t -> p ic t", p=P
                    ),
                )
                for tq in range(2):
                    tc512 = half * 2 + tq
                    pm = mpool.tile([P, 512], f32)
                    for ic in range(IC):
                        nc.tensor.matmul(
                            pm[:],
                            lhsT=wt_sb[:, ic, :],
                            rhs=xt[:, ic, tq * 512 : (tq + 1) * 512],
                            start=(ic == 0),
                            stop=(ic == IC - 1),
                        )
                    idx = b * (T // 512) + tc512
                    dst = wxbuf[:, b, tc512 * 512 : (tc512 + 1) * 512]
                    nc.scalar.activation(
                        dst, pm[:], Act.Identity,
                        accum_out=sumS[:, idx : idx + 1],
                    )
                    trash = scr.tile([P, 512], f32, tag="trash", name="trash")
                    nc.vector.scalar_tensor_tensor(
                        trash[:],
                        dst, 1.0, dst,
                        op0=Alu.bypass, op1=Alu.mult,
                        accum_out=sumQ[:, idx : idx + 1],
                    )

        # ---- phase B: stats allreduce (pairs), BN finalize, e-transform ----
        stats = small.tile([P, 2], f32)
        nc.vector.tensor_reduce(
            stats[:, 0:1], sumS[:], axis=mybir.AxisListType.X, op=Alu.add
        )
        nc.vector.tensor_reduce(
            stats[:, 1:2], sumQ[:], axis=mybir.AxisListType.X, op=Alu.add
        )
        cc_in = dram.tile([P, 2], f32)
        cc_out = dram.tile([P, 2], f32)
        nc.sync.dma_start(cc_in[:], stats[:])
        nc.gpsimd.collective_compute(
            "AllReduce",
            Alu.add,
            replica_groups=[[0, 1], [2, 3], [4, 5], [6, 7]],
            ins=[cc_in.opt()],
            outs=[cc_out.opt()],
        )
        gstats = small.tile([P, 2], f32)
        nc.sync.dma_start(gstats[:], cc_out[:])

        mean = small.tile([P, 1], f32)
        ex2 = small.tile([P, 1], f32)
        var = small.tile([P, 1], f32)
        inv = small.tile([P, 1], f32)
        gsc = small.tile([P, 1], f32)
        hof = small.tile([P, 1], f32)
        tmp = small.tile([P, 1], f32)
        nc.vector.tensor_scalar(mean[:], gstats[:, 0:1], 1.0 / NTOT, None, op0=Alu.mult)
        nc.vector.tensor_scalar(ex2[:], gstats[:, 1:2], 1.0 / NTOT, None, op0=Alu.mult)
        nc.vector.tensor_tensor(tmp[:], mean[:], mean[:], op=Alu.mult)
        nc.vector.tensor_tensor(var[:], ex2[:], tmp[:], op=Alu.subtract)
        nc.vector.tensor_scalar(var[:], var[:], 1e-5, None, op0=Alu.add)
        nc.scalar.sqrt(tmp[:], var[:])
        nc.vector.reciprocal(inv[:], tmp[:])
        nc.vector.tensor_tensor(gsc[:], bg, inv[:], op=Alu.mult)
        nc.vector.tensor_tensor(tmp[:], mean[:], gsc[:], op=Alu.mult)
        nc.vector.tensor_tensor(hof[:], bb, tmp[:], op=Alu.subtract)

        # ---- BN-apply chunks (in-place, Act only): d_j = gsc*Wx_j + hof ----
        DCH = 128
        ndch = T // DCH

        def issue_dchunk(k):
            c0 = k * DCH
            nc.scalar.activation(
                wxbuf[:, :, c0 : c0 + DCH], wxbuf[:, :, c0 : c0 + DCH],
                Act.Identity, bias=hof[:], scale=gsc[:],
            )

        for k in range(5):
            issue_dchunk(k)

        # ---- initial state ----
        # cw_0 = w_{-1} + d_0 ; r_0 = -a_r*negm_0 + cw_0
        # w_0 = aisq*negm_0 + a_r*w_{-1}
        cw0 = state.tile([P, BLOC], f32, tag="cw0")
        nc.gpsimd.tensor_tensor(cw0[:], st0_sb[:, 1], wxbuf[:, :, 0], op=Alu.add)
        r0t = state.tile([P, BLOC], f32, tag="r0t")
        nc.vector.scalar_tensor_tensor(
            r0t[:], st0_sb[:, 0], nar, cw0[:], op0=Alu.mult, op1=Alu.add
        )
        w_t = [
            state.tile([P, BLOC], f32, tag=f"w{i}", name=f"w{i}")
            for i in range(3)
        ]
        cw_t = [
            state.tile([P, BLOC], f32, tag=f"cw{i}", name=f"cw{i}")
            for i in range(3)
        ]
        y0 = state.tile([P, BLOC], f32, tag="y0")
        nc.vector.tensor_scalar(y0[:], st0_sb[:, 1], arr, None, op0=Alu.mult)
        nc.vector.scalar_tensor_tensor(
            w_t[0][:], st0_sb[:, 0], aisq, y0[:], op0=Alu.mult, op1=Alu.add
        )

        # ---- phase C: w-form recurrence ----
        #   DVE:  w_t = ((r_{t-1}>.5)-r_{t-1})*aisq + a_r*w_{t-1}   [issued 1st]
        #   DVE:  r_t = ((r_{t-1}>.5)-r_{t-1})*(-a_r) + cw_t      -> col t-1
        #   Pool: cw_{t+1} = w_t + d_{t+1}  (one step ahead so the r-op
        #         never waits on the cross-engine add)
        nT = TSTEPS
        nc.gpsimd.tensor_tensor(
            cw_t[1][:], w_t[0][:], wxbuf[:, :, 1], op=Alu.add
        )
        for t in range(1, nT):
            wp = w_t[(t - 1) % 3][:]
            wn = w_t[t % 3][:]
            rm1 = r0t[:] if t == 1 else wxbuf[:, :, t - 2]
            if t + 1 < nT:
                nc.vector._custom_dve(
                    LIFG, out=wn, in0=rm1, in1=wp,
                    s0=aisq, s1=arr, imm2=0.5,
                )
                nc.gpsimd.tensor_tensor(
                    cw_t[(t + 1) % 3][:], wn, wxbuf[:, :, t + 1], op=Alu.add
                )
            nc.vector._custom_dve(
                LIFG, out=wxbuf[:, :, t - 1], in0=rm1, in1=cw_t[t % 3][:],
                s0=nar, s1=1.0, imm2=0.5,
            )
            # keep BN-apply four chunks ahead of the consumer
            if t % DCH == 0 and t // DCH + 4 < ndch:
                issue_dchunk(t // DCH + 4)
            # cols [t-OCH, t) are final once r_t (col t-1) is issued
            if t >= OCH and t % OCH == 0 and t + OCH <= nT:
                nc.sync.dma_start(
                    out_d[:, :, t - OCH : t], wxbuf[:, :, t - OCH : t]
                )
        nc.sync.dma_start(out0_d[:], r0t[:])
        if nT == T:
            nc.sync.dma_start(out_d[:, :, T - OCH :], wxbuf[:, :, T - OCH :])
        else:  # dev truncation: dump everything in one go
            nc.sync.dma_start(out_d[:], wxbuf[:])

    nc.compile()
    return nc


def _prep_host(W, log_log_alpha, log_dt, alpha_img, b, gamma, beta):
    lla = np.exp(log_log_alpha.astype(np.float32))
    dtv = np.exp(log_dt.astype(np.float32)).astype(np.float32)
    z = (-lla.astype(np.complex64) + 1j * alpha_img.astype(np.complex64)) * dtv
    alpha = np.exp(z.astype(np.complex64))
    a_r = alpha.real.astype(np.float32)
    a_i = alpha.imag.astype(np.float32)
    bgv = (b * gamma).astype(np.float32)
    bbv = (b * beta).astype(np.float32)
    return a_r, a_i, bgv, bbv


def kernel(x, W, log_log_alpha, log_dt, alpha_img, b, gamma, beta,
           u0_real, u0_imag, s0):
    from concourse.bass_utils import run_bass_kernel_spmd

    if "nc" not in _CACHE:
        _CACHE["nc"] = _build()
    nc = _CACHE["nc"]

    a_r, a_i, bgv, bbv = _prep_host(
        W, log_log_alpha, log_dt, alpha_img, b, gamma, beta
    )
    negm0 = (s0 - u0_real).astype(np.float32)       # [B, H]
    winit = (-a_i[None, :] * u0_imag).astype(np.float32)

    xf = np.asarray(x, dtype=np.float32)
    xT = [
        np.ascontiguousarray(xf[h * BLOC : (h + 1) * BLOC].transpose(0, 2, 1))
        for h in range(2)
    ]

    in_maps = []
    for c in range(NCORES):
        bh = c % 2
        hq = c // 2
        hs = slice(hq * HLOC, (hq + 1) * HLOC)
        bs = slice(bh * BLOC, (bh + 1) * BLOC)
        wt = np.ascontiguousarray(
            W[hs, :].T.reshape(IC, P, HLOC).transpose(1, 0, 2)
        ).astype(np.float32)
        cons = np.stack(
            [-a_r[hs], a_r[hs], (a_i * a_i)[hs], bgv[hs], bbv[hs],
             (a_r * a_r + a_i * a_i)[hs]], axis=1
        ).astype(np.float32)
        st0 = np.stack([negm0[bs, hs].T, winit[bs, hs].T], axis=1).astype(
            np.float32
        )
        in_maps.append({
            "x": xT[bh],
            "wt": wt,
            "cons": np.ascontiguousarray(cons),
            "st0": np.ascontiguousarray(st0),
        })

    res = run_bass_kernel_spmd(
        nc,
        in_maps,
        core_ids=list(range(NCORES)),
        trace=bool(int(os.environ.get("LIF_TRACE", "0"))),
    )
    _CACHE["last_res"] = res

    out = np.empty((B, T, H), np.float32)
    for c in range(NCORES):
        bh = c % 2
        hq = c // 2
        o = res.results[c]["out"]              # [128h, 16b, 2048cols]
        o0 = res.results[c]["out0"]            # [128h, 16b] = r_0
        # col j holds r_{j+1} for j < T-1; r_0 arrives separately
        r = np.concatenate([o0[:, :, None], o[:, :, : T - 1]], axis=2)
        s = (r > 0.5).astype(np.float32)
        out[bh * BLOC : (bh + 1) * BLOC, :, hq * HLOC : (hq + 1) * HLOC] = (
            s.transpose(1, 2, 0)
        )
    return out


# revision 4
# speedup vs baseline: 1.0123x; 1.0123x over previous
"""Trainium2 Bass kernel for nn_LIFcomplexLayer (v2).

Sharding: (B x H) = 2 x 4 across 8 cores: core c handles batches
[(c%2)*16, +16) and neurons [(c//2)*128, +128). With only 128 neurons per
core, every per-neuron constant is a per-partition [P,1] scalar, so the
serial recurrence runs as 4 fused scalar_tensor_tensor ops per step
(2 on Pool, 2 on DVE) over flat [128,16] tiles.

Recurrence algebra (alpha = a_r + i*a_i, state w = -a_i*ui, v = w + d_next):
  negm_t = (r_{t-1} > 0.5) - r_{t-1}          (= s_{t-1} - r_{t-1})
  r_t    = -a_r * negm_t + v_{t-1}
  v_t    = a_i^2 * negm_t + a_r * v_{t-1} + e_t
  e_t    = d_{t+1} - a_r * d_t               (precomputed in bulk)
where d = BN(Wx)*gamma*b + beta*b is folded as
  e_t = gsc*(Wx_{t+1} - a_r*Wx_t) + hof*(1 - a_r),
  gsc = b*gamma*invstd, hof = b*beta - mean*gsc.

Phase A: host pre-transposes x to [b, i, t]; device streams it and runs
fp32r matmuls (1 cyc/row) against W^T, accumulating raw Wx in a resident
SBUF buffer [128h, 16b, 2048t] plus per-neuron sum/sumsq for BN.
Phase B: stats AllReduce across the 2 cores sharing each h-slice, BN
finalize, chunked in-place e-transform, v_{-1} init.
Phase C: 2048-step loop; r_t is written into wxbuf column (t-1) mod 2048
(the e column consumed one step earlier), raw membrane values are DMA'd
out in 256-col chunks overlapping the loop; host applies the spike
threshold (bit-identical in f32) and un-rotates the column mapping.
"""

import sys

if "/opt/trn_rl_repo" not in sys.path:
    sys.path.insert(0, "/opt/trn_rl_repo")

import os
import numpy as np

B, T, I, H = 32, 2048, 512, 512
NCORES = 8
P = 128
BLOC = 16                   # batches per core
HLOC = 128                  # neurons per core
IC = I // P                 # 4 i-chunks
NTOT = float(B * T)
ECH = 32                    # e-transform chunk width (16b x 32 = 512 PSUM f32)
OCH = 256                   # output DMA chunk width

TSTEPS = int(os.environ.get("LIF_TSTEPS", str(T)))

_CACHE = {}


def _register_lif_ops():
    import numpy as np
    from concourse.dve_spec import Spec, Src0, Src1, C0, C1, lower, _has_src1
    from concourse.dve_uop import DveOpSpec
    from concourse.dve_ops import (
        DveOp, OPS, CUSTOM_DVE_SPECS, _SUB_OPCODE_FOR_NAME,
        _CUSTOM_DVE_ROW_BASE,
    )

    for op in OPS:
        if op.name == "LIFG_ANT":
            return op

    def mk(name, body, ref):
        spec = Spec(body=body, reference=ref)
        row = _CUSTOM_DVE_ROW_BASE + len(OPS)
        uops = lower(spec, ver="v3")
        tmp = DveOpSpec(name=name, opcode=row, uops=uops, rd1_en=_has_src1(spec))
        sha = tmp.sha("v3")
        op = DveOp(name, spec, subdim=False, uops_sha={"v3": sha})
        OPS.append(op)
        CUSTOM_DVE_SPECS[name] = spec
        _SUB_OPCODE_FOR_NAME[name] = row
        return op

    from concourse.dve_spec import C2
    s0gt = Src0 > C2
    lifg = mk(
        "LIFG_ANT", (s0gt - Src0) * C0 + Src1 * C1,
        lambda in0, in1, s0, s1, imm2:
            (((in0 > imm2).astype(np.float32)) - in0) * s0 + in1 * s1,
    )
    return lifg


def _build():
    import concourse.bass as bass
    import concourse.bacc as bacc
    import concourse.tile as tile
    from concourse import mybir
    from contextlib import ExitStack

    dt = mybir.dt
    f32 = dt.float32
    Alu = mybir.AluOpType
    Act = mybir.ActivationFunctionType

    LIFG = _register_lif_ops()

    nc = bacc.Bacc(
        "TRN2", target_bir_lowering=False, debug=False, num_devices=NCORES
    )

    # x pre-transposed on host: [b, i, t]
    x_d = nc.dram_tensor("x", [BLOC, I, T], f32, kind="ExternalInput").ap()
    wt_d = nc.dram_tensor("wt", [P, IC, HLOC], f32, kind="ExternalInput").ap()
    cons_d = nc.dram_tensor("cons", [P, 6], f32, kind="ExternalInput").ap()
    st0_d = nc.dram_tensor("st0", [P, 2, BLOC], f32, kind="ExternalInput").ap()
    out_d = nc.dram_tensor("out", [P, BLOC, T], f32, kind="ExternalOutput").ap()
    out0_d = nc.dram_tensor("out0", [P, BLOC], f32, kind="ExternalOutput").ap()

    with tile.TileContext(nc) as tc, ExitStack() as ctx:
        consts = ctx.enter_context(tc.tile_pool(name="consts", bufs=1))
        big = ctx.enter_context(tc.tile_pool(name="big", bufs=1))
        xin = ctx.enter_context(tc.tile_pool(name="xin", bufs=2))
        mpool = ctx.enter_context(tc.tile_pool(name="psumM", bufs=4, space="PSUM"))
        small = ctx.enter_context(tc.tile_pool(name="small", bufs=1))
        scr = ctx.enter_context(tc.tile_pool(name="scr", bufs=2))
        state = ctx.enter_context(tc.tile_pool(name="state", bufs=1))
        dram = ctx.enter_context(tc.tile_pool(name="dram", bufs=1, space="DRAM"))

        wt_sb = consts.tile([P, IC, HLOC], f32)
        nc.sync.dma_start(wt_sb[:], wt_d[:])
        cons_sb = consts.tile([P, 6], f32)
        nc.sync.dma_start(cons_sb[:], cons_d[:])
        st0_sb = consts.tile([P, 2, BLOC], f32)
        nc.sync.dma_start(st0_sb[:], st0_d[:])

        nar = cons_sb[:, 0:1]
        arr = cons_sb[:, 1:2]
        aisq = cons_sb[:, 2:3]
        bg = cons_sb[:, 3:4]
        bb = cons_sb[:, 4:5]
        aa2 = cons_sb[:, 5:6]

        # Raw Wx, then e, then membrane values r (shifted one column).
        wxbuf = big.tile([P, BLOC, T], f32)
        NST = BLOC * (T // 512)  # stat columns: one per (b, 512-t chunk)
        sumS = small.tile([P, NST], f32)
        sumQ = small.tile([P, NST], f32)

        # ---- phase A: stream xT, fp32r matmuls, raw Wx + stats ----
        for b in range(BLOC):
            for half in range(2):
                xt = xin.tile([P, IC, T // 2], f32)
                nc.sync.dma_start(
                    xt[:],
                    x_d[b, :, half * (T // 2) : (half + 1) * (T // 2)].rearrange(
                        "(ic p) t -> p ic t", p=P
                    ),
                )
                for tq in range(2):
                    tc512 = half * 2 + tq
                    pm = mpool.tile([P, 512], f32)
                    for ic in range(IC):
                        nc.tensor.matmul(
                            pm[:],
                            lhsT=wt_sb[:, ic, :],
                            rhs=xt[:, ic, tq * 512 : (tq + 1) * 512],
                            start=(ic == 0),
                            stop=(ic == IC - 1),
                        )
                    idx = b * (T // 512) + tc512
                    dst = wxbuf[:, b, tc512 * 512 : (tc512 + 1) * 512]
                    nc.scalar.activation(
                        dst, pm[:], Act.Identity,
                        accum_out=sumS[:, idx : idx + 1],
                    )
                    trash = scr.tile([P, 512], f32, tag="trash", name="trash")
                    nc.vector.scalar_tensor_tensor(
                        trash[:],
                        dst, 1.0, dst,
                        op0=Alu.bypass, op1=Alu.mult,
                        accum_out=sumQ[:, idx : idx + 1],
                    )

        # ---- phase B: stats allreduce (pairs), BN finalize, e-transform ----
        stats = small.tile([P, 2], f32)
        nc.vector.tensor_reduce(
            stats[:, 0:1], sumS[:], axis=mybir.AxisListType.X, op=Alu.add
        )
        nc.vector.tensor_reduce(
            stats[:, 1:2], sumQ[:], axis=mybir.AxisListType.X, op=Alu.add
        )
        cc_in = dram.tile([P, 2], f32)
        cc_out = dram.tile([P, 2], f32)
        nc.sync.dma_start(cc_in[:], stats[:])
        nc.gpsimd.collective_compute(
            "AllReduce",
            Alu.add,
            replica_groups=[[0, 1], [2, 3], [4, 5], [6, 7]],
            ins=[cc_in.opt()],
            outs=[cc_out.opt()],
        )
        gstats = small.tile([P, 2], f32)
        nc.sync.dma_start(gstats[:], cc_out[:])

        mean = small.tile([P, 1], f32)
        ex2 = small.tile([P, 1], f32)
        var = small.tile([P, 1], f32)
        inv = small.tile([P, 1], f32)
        gsc = small.tile([P, 1], f32)
        hof = small.tile([P, 1], f32)
        tmp = small.tile([P, 1], f32)
        nc.vector.tensor_scalar(mean[:], gstats[:, 0:1], 1.0 / NTOT, None, op0=Alu.mult)
        nc.vector.tensor_scalar(ex2[:], gstats[:, 1:2], 1.0 / NTOT, None, op0=Alu.mult)
        nc.vector.tensor_tensor(tmp[:], mean[:], mean[:], op=Alu.mult)
        nc.vector.tensor_tensor(var[:], ex2[:], tmp[:], op=Alu.subtract)
        nc.vector.tensor_scalar(var[:], var[:], 1e-5, None, op0=Alu.add)
        nc.scalar.sqrt(tmp[:], var[:])
        nc.vector.reciprocal(inv[:], tmp[:])
        nc.vector.tensor_tensor(gsc[:], bg, inv[:], op=Alu.mult)
        nc.vector.tensor_tensor(tmp[:], mean[:], gsc[:], op=Alu.mult)
        nc.vector.tensor_tensor(hof[:], bb, tmp[:], op=Alu.subtract)

        # ---- BN-apply chunks (in-place, Act only): d_j = gsc*Wx_j + hof ----
        DCH = 128
        ndch = T // DCH

        def issue_dchunk(k):
            c0 = k * DCH
            nc.scalar.activation(
                wxbuf[:, :, c0 : c0 + DCH], wxbuf[:, :, c0 : c0 + DCH],
                Act.Identity, bias=hof[:], scale=gsc[:],
            )

        for k in range(5):
            issue_dchunk(k)

        # ---- initial state ----
        # cw_0 = w_{-1} + d_0 ; r_0 = -a_r*negm_0 + cw_0
        # w_0 = aisq*negm_0 + a_r*w_{-1}
        cw0 = state.tile([P, BLOC], f32, tag="cw0")
        nc.gpsimd.tensor_tensor(cw0[:], st0_sb[:, 1], wxbuf[:, :, 0], op=Alu.add)
        r0t = state.tile([P, BLOC], f32, tag="r0t")
        nc.vector.scalar_tensor_tensor(
            r0t[:], st0_sb[:, 0], nar, cw0[:], op0=Alu.mult, op1=Alu.add
        )
        w_t = [
            state.tile([P, BLOC], f32, tag=f"w{i}", name=f"w{i}")
            for i in range(3)
        ]
        cw_t = [
            state.tile([P, BLOC], f32, tag=f"cw{i}", name=f"cw{i}")
            for i in range(3)
        ]
        y0 = state.tile([P, BLOC], f32, tag="y0")
        nc.vector.tensor_scalar(y0[:], st0_sb[:, 1], arr, None, op0=Alu.mult)
        nc.vector.scalar_tensor_tensor(
            w_t[0][:], st0_sb[:, 0], aisq, y0[:], op0=Alu.mult, op1=Alu.add
        )

        # ---- phase C: w-form recurrence ----
        #   DVE:  w_t = ((r_{t-1}>.5)-r_{t-1})*aisq + a_r*w_{t-1}   [issued 1st]
        #   DVE:  r_t = ((r_{t-1}>.5)-r_{t-1})*(-a_r) + cw_t      -> col t-1
        #   Pool: cw_{t+1} = w_t + d_{t+1}  (one step ahead so the r-op
        #         never waits on the cross-engine add)
        nT = TSTEPS
        nc.gpsimd.tensor_tensor(
            cw_t[1][:], w_t[0][:], wxbuf[:, :, 1], op=Alu.add
        )
        for t in range(1, nT):
            wp = w_t[(t - 1) % 3][:]
            wn = w_t[t % 3][:]
            rm1 = r0t[:] if t == 1 else wxbuf[:, :, t - 2]
            if t + 1 < nT:
                nc.vector._custom_dve(
                    LIFG, out=wn, in0=rm1, in1=wp,
                    s0=aisq, s1=arr, imm2=0.5,
                )
                nc.gpsimd.tensor_tensor(
                    cw_t[(t + 1) % 3][:], wn, wxbuf[:, :, t + 1], op=Alu.add
                )
            nc.vector._custom_dve(
                LIFG, out=wxbuf[:, :, t - 1], in0=rm1, in1=cw_t[t % 3][:],
                s0=nar, s1=1.0, imm2=0.5,
            )
            # keep BN-apply four chunks ahead of the consumer
            if t % DCH == 0 and t // DCH + 4 < ndch:
                issue_dchunk(t // DCH + 4)
            # cols [t-OCH, t) are final once r_t (col t-1) is issued
            if t >= OCH and t % OCH == 0 and t + OCH <= nT:
                nc.sync.dma_start(
                    out_d[:, :, t - OCH : t], wxbuf[:, :, t - OCH : t]
                )
            elif nT == T and t == T - 64:
                nc.sync.dma_start(
                    out_d[:, :, T - OCH : T - 64], wxbuf[:, :, T - OCH : T - 64]
                )
        nc.sync.dma_start(out0_d[:], r0t[:])
        if nT == T:
            nc.sync.dma_start(out_d[:, :, T - 64 :], wxbuf[:, :, T - 64 :])
        else:  # dev truncation: dump everything in one go
            nc.sync.dma_start(out_d[:], wxbuf[:])

    nc.compile()
    return nc


def _prep_host(W, log_log_alpha, log_dt, alpha_img, b, gamma, beta):
    lla = np.exp(log_log_alpha.astype(np.float32))
    dtv = np.exp(log_dt.astype(np.float32)).astype(np.float32)
    z = (-lla.astype(np.complex64) + 1j * alpha_img.astype(np.complex64)) * dtv
    alpha = np.exp(z.astype(np.complex64))
    a_r = alpha.real.astype(np.float32)
    a_i = alpha.imag.astype(np.float32)
    bgv = (b * gamma).astype(np.float32)
    bbv = (b * beta).astype(np.float32)
    return a_r, a_i, bgv, bbv


def kernel(x, W, log_log_alpha, log_dt, alpha_img, b, gamma, beta,
           u0_real, u0_imag, s0):
    from concourse.bass_utils import run_bass_kernel_spmd

    if "nc" not in _CACHE:
        _CACHE["nc"] = _build()
    nc = _CACHE["nc"]

    a_r, a_i, bgv, bbv = _prep_host(
        W, log_log_alpha, log_dt, alpha_img, b, gamma, beta
    )
    negm0 = (s0 - u0_real).astype(np.float32)       # [B, H]
    winit = (-a_i[None, :] * u0_imag).astype(np.float32)

    xf = np.asarray(x, dtype=np.float32)
    xT = [
        np.ascontiguousarray(xf[h * BLOC : (h + 1) * BLOC].transpose(0, 2, 1))
        for h in range(2)
    ]

    in_maps = []
    for c in range(NCORES):
        bh = c % 2
        hq = c // 2
        hs = slice(hq * HLOC, (hq + 1) * HLOC)
        bs = slice(bh * BLOC, (bh + 1) * BLOC)
        wt = np.ascontiguousarray(
            W[hs, :].T.reshape(IC, P, HLOC).transpose(1, 0, 2)
        ).astype(np.float32)
        cons = np.stack(
            [-a_r[hs], a_r[hs], (a_i * a_i)[hs], bgv[hs], bbv[hs],
             (a_r * a_r + a_i * a_i)[hs]], axis=1
        ).astype(np.float32)
        st0 = np.stack([negm0[bs, hs].T, winit[bs, hs].T], axis=1).astype(
            np.float32
        )
        in_maps.append({
            "x": xT[bh],
            "wt": wt,
            "cons": np.ascontiguousarray(cons),
            "st0": np.ascontiguousarray(st0),
        })

    res = run_bass_kernel_spmd(
        nc,
        in_maps,
        core_ids=list(range(NCORES)),
        trace=bool(int(os.environ.get("LIF_TRACE", "0"))),
    )
    _CACHE["last_res"] = res

    out = np.empty((B, T, H), np.float32)
    for c in range(NCORES):
        bh = c % 2
        hq = c // 2
        o = res.results[c]["out"]              # [128h, 16b, 2048cols]
        o0 = res.results[c]["out0"]            # [128h, 16b] = r_0
        # col j holds r_{j+1} for j < T-1; r_0 arrives separately
        r = np.concatenate([o0[:, :, None], o[:, :, : T - 1]], axis=2)
        s = (r > 0.5).astype(np.float32)
        out[bh * BLOC : (bh + 1) * BLOC, :, hq * HLOC : (hq + 1) * HLOC] = (
            s.transpose(1, 2, 0)
        )
    return out
